# revision 7
# baseline (speedup 1.0000x reference)
"""CrossAttentionHead TRN2 kernel (v5).

Full inputs -> full output. Shards batch (B=8) across 8 NeuronCores,
one batch element per core (pure data parallel, no collectives).

Per-core layout (xT staged host-side as bf16 [E=768, S=2048]):
  qT/kT/vT = W*.T @ xT + b*          ([H=128, S], weights stationary, bf16)
  vN       = blockwise transpose(vT)  ([S,H] natural, bf16)
  scores   sT[sk, sq] = kT_blk.T @ qT (psum f32, 1024-sq halves, ring of 2)
  es       = exp(sT * 1/sqrt(768))    (ScalarE, bf16 out)
  acc     += es                       (DVE bf16, for row sums)
  oT      += vN_blk.T @ es            (PV accumulate, [H, S] psum f32)
  rowsumT  = acc_blk.T @ ones         (PE, [sq,1] per 128-block -> [128,16])
  out      = transpose(oT) * (1/rowsumT)  (wide DVE mul, broadcast scale)

Critical path is the xT DMA stream (~17 us at observed ~215 GB/s,
cannot start before the ~8.5 us DGE bring-up) followed by the
ScalarE-bound exp stream (4.19M elems ~ 34 us). Everything else hides:
q,k projections run chunk-outer chasing the DMA; the v projection, vN
transposes, and the attention*V matmuls are interleaved into the main
loop's PE slack (PE needs ~1.7 us of a ~2.1 us scalar-bound period);
drains are sliced across ScalarE/DVE. The s-psum ring (2 x [128,1024])
plus the oT accumulator [128,2048] exactly fill the 8 PSUM banks while
the v psum borrows the other side's banks early in the loop.

Matmuls stream 1 cyc/col at 2.4 GHz regardless of bf16/f32r; bf16 is
for halved DMA/SBUF traffic and 2x DVE rates. Softmax skips
max-subtraction: energy/sqrt(768) ~ N(0, 0.41^2).
Measured numerics vs fp32 reference: rel err ~6e-3 (gate 2e-2).
"""

import sys

if '/opt/trn_rl_repo' not in sys.path:
    sys.path.insert(0, '/opt/trn_rl_repo')

import numpy as np

B, S, E, H = 8, 2048, 768, 128
NCORES = 8
ST = S // 128           # 16 sk tiles
EC = E // 128           # 6 embed chunks
SCALE = float(1.0 / np.sqrt(np.float32(E)))

_CACHE = {}


def _build():
    import concourse.bacc as bacc
    import concourse.mybir as mybir
    import concourse.tile as tile

    dt = mybir.dt
    f32 = dt.float32
    bf16 = dt.bfloat16
    AF = mybir.ActivationFunctionType

    nc = bacc.Bacc(None, target_bir_lowering=False)
    xT_d = nc.dram_tensor("xT", [E, S], dt.uint16, kind="ExternalInput")
    idb_d = nc.dram_tensor("identb", [128, 128], dt.uint16,
                           kind="ExternalInput")
    w_d = {}
    b_d = {}
    for nm in ("q", "k", "v"):
        w_d[nm] = nc.dram_tensor(f"W{nm}", [E, H], dt.uint16,
                                 kind="ExternalInput")
        b_d[nm] = nc.dram_tensor(f"b{nm}", [H], f32, kind="ExternalInput")
    out_d = nc.dram_tensor("out", [S, H], f32, kind="ExternalOutput")

    with tile.TileContext(nc) as tc:
        with tc.tile_pool(name="data", bufs=1) as db, \
             tc.tile_pool(name="es", bufs=8) as esp:
            identb = db.tile([128, 128], bf16, name="identb")
            onesb = db.tile([128, 1], bf16, name="onesb")
            nc.vector.memset(onesb[:], 1.0)

            # DMA plan: x chunks alternate sync/scalar with nothing ahead
            # of them; weights + identity ride the gpsimd queue (needed a
            # few us later than chunk0); biases trail on sync.
            xT = [db.tile([128, S], bf16, name=f"xT{c}") for c in range(EC)]
            w_sb = {}
            b_sb = {}
            for nm in ("q", "k", "v"):
                w_sb[nm] = db.tile([128, EC, H], bf16, name=f"w_{nm}")

            for c in range(EC):
                eng = nc.sync if c % 2 == 0 else nc.scalar
                eng.dma_start(
                    out=xT[c][:],
                    in_=xT_d[c * 128:(c + 1) * 128, :].bitcast(bf16))
            for nm in ("q", "k", "v"):
                nc.gpsimd.dma_start(
                    out=w_sb[nm][:],
                    in_=w_d[nm].rearrange("(c p) d -> p c d", p=128)
                    .bitcast(bf16))
            nc.gpsimd.dma_start(out=identb[:], in_=idb_d[:, :].bitcast(bf16))
            for nm in ("q", "k", "v"):
                b_sb[nm] = db.tile([128, 1], f32, name=f"b_{nm}")
                nc.sync.dma_start(out=b_sb[nm][:], in_=b_d[nm][:, None])

            qT = db.tile([128, S], bf16, name="qT")
            kT = db.tile([128, S], bf16, name="kT")
            vT = db.tile([128, S], bf16, name="vT")
            vN = db.tile([128, S], bf16, name="vN")
            acc = db.tile([128, S], bf16, name="acc")
            oT_sb = db.tile([128, S], bf16, name="oT_sb")
            rcpT = db.tile([128, ST], f32, name="rcpT")
            stage = db.tile([128, S], f32, name="stage")

            # ---- warm-up: ramp the PE clock while DMAs stream ----
            wj = db.tile([128, 512], bf16, name="wjunk")
            nc.vector.memset(wj[:], 0.03125)
            with tc.tile_pool(name="pw", bufs=1, space="PSUM",
                              side="left") as pw:
                wps = pw.tile([128, 512], f32, tag="w")
                for _ in range(9):
                    nc.tensor.matmul(wps[:], wj[:, :128], wj[:],
                                     start=True, stop=True)
                wsink = db.tile([128, 512], f32, name="wsink")
                nc.vector.tensor_copy(wsink[:], wps[:])

            # ---- q,k projections chunk-outer chasing the DMA stream ----
            pq_cm = tc.tile_pool(name="pq", bufs=1, space="PSUM", side="left")
            pq = pq_cm.__enter__()
            q_ps = pq.tile([128, S], f32, tag="q")
            pk_cm = tc.tile_pool(name="pk", bufs=1, space="PSUM", side="right")
            pk = pk_cm.__enter__()
            k_ps = pk.tile([128, S], f32, tag="k")
            for c in range(EC):
                for ps in (q_ps, k_ps):
                    nm = "q" if ps is q_ps else "k"
                    for n in range(4):
                        nc.tensor.matmul(
                            ps[:, n * 512:(n + 1) * 512],
                            w_sb[nm][:, c, :],
                            xT[c][:, n * 512:(n + 1) * 512],
                            start=(c == 0), stop=(c == EC - 1))

            def drain(dst, src, bias, n):
                sl = slice(n * 512, (n + 1) * 512)
                if n % 2 == 0:
                    nc.scalar.activation(dst[:, sl], src[:, sl], AF.Identity,
                                         bias=bias[:], scale=1.0)
                else:
                    nc.vector.tensor_scalar_add(dst[:, sl], src[:, sl],
                                                bias[:])

            for n in range(4):
                drain(qT, q_ps, b_sb["q"], n)
            pq_cm.__exit__(None, None, None)

            ps_cm = tc.tile_pool(name="ps", bufs=2, space="PSUM", side="left")
            psl = ps_cm.__enter__()

            for n in range(4):
                drain(kT, k_ps, b_sb["k"], n)
            pk_cm.__exit__(None, None, None)

            pv_cm = tc.tile_pool(name="pv", bufs=1, space="PSUM", side="right")
            pv = pv_cm.__enter__()
            v_ps = pv.tile([128, S], f32, tag="v")

            def emit_scores(kt, h):
                t = psl.tile([128, 1024], f32, tag="s")
                for n in range(2):
                    q0 = h * 1024 + n * 512
                    nc.tensor.matmul(
                        t[:, n * 512:(n + 1) * 512],
                        kT[:, kt * 128:(kt + 1) * 128],
                        qT[:, q0:q0 + 512],
                        start=True, stop=True)
                return t

            def emit_v_chunk(c):
                for n in range(4):
                    nc.tensor.matmul(
                        v_ps[:, n * 512:(n + 1) * 512],
                        w_sb["v"][:, c, :],
                        xT[c][:, n * 512:(n + 1) * 512],
                        start=(c == 0), stop=(c == EC - 1))

            def emit_av(kt, es):
                for n in range(4):
                    nc.tensor.matmul(
                        oT_ps[:, n * 512:(n + 1) * 512],
                        vN[:, kt * 128:(kt + 1) * 128],
                        es[:, n * 512:(n + 1) * 512],
                        start=(kt == 0), stop=(kt == ST - 1))

            # main loop: the exp stream paces everything; v projection,
            # vN transposes and lagging AV matmuls ride the PE slack.
            s_half = [emit_scores(0, 0), emit_scores(0, 1)]
            es_t = {}
            av_pending = []
            pvnt_cm = None
            poT_cm = None
            oT_ps = None

            for kt in range(ST):
                es = esp.tile([128, S], bf16, tag="es")
                es_t[kt] = es
                for h in range(2):
                    nc.scalar.activation(
                        es[:, h * 1024:(h + 1) * 1024], s_half[h][:],
                        AF.Exp, scale=SCALE)
                if kt == 0:
                    nc.vector.tensor_copy(acc[:], es[:])
                else:
                    nc.vector.tensor_add(acc[:], acc[:], es[:])
                av_pending.append(kt)
                if kt < ST - 1:
                    s_half = [emit_scores(kt + 1, 0), emit_scores(kt + 1, 1)]
                # staged extra PE work per period
                if kt == 0:
                    emit_v_chunk(0)
                    emit_v_chunk(1)
                elif kt == 1:
                    emit_v_chunk(2)
                    emit_v_chunk(3)
                elif kt == 2:
                    emit_v_chunk(4)
                    emit_v_chunk(5)
                elif kt == 3:
                    # v drains (engines reach these after exp0-3 / acc0-3)
                    for n in range(4):
                        drain(vT, v_ps, b_sb["v"], n)
                    pv_cm.__exit__(None, None, None)
                elif kt == 4:
                    pvnt_cm = tc.tile_pool(name="pvnt", bufs=2, space="PSUM",
                                           side="right")
                    pvnt = pvnt_cm.__enter__()
                    for j in range(ST):
                        pt = pvnt.tile([128, 128], bf16, tag="vt")
                        nc.tensor.transpose(
                            pt[:], vT[:, j * 128:(j + 1) * 128], identb[:])
                        nc.vector.tensor_copy(
                            vN[:, j * 128:(j + 1) * 128], pt[:])
                    pvnt_cm.__exit__(None, None, None)
                    poT_cm = tc.tile_pool(name="poT", bufs=1, space="PSUM",
                                          side="right")
                    poT = poT_cm.__enter__()
                    oT_ps = poT.tile([128, S], f32, tag="o")
                elif kt >= 5:
                    navs = 2 if kt % 2 == 1 else 1
                    for _ in range(navs):
                        if av_pending and av_pending[0] <= kt - 1:
                            j = av_pending.pop(0)
                            emit_av(j, es_t[j])
            while av_pending:
                j = av_pending.pop(0)
                emit_av(j, es_t[j])
            ps_cm.__exit__(None, None, None)

            # ---- finale ----
            pf_cm = tc.tile_pool(name="pf", bufs=1, space="PSUM", side="left")
            pf = pf_cm.__enter__()
            rsT_ps = pf.tile([128, ST], f32, tag="rs")
            for j in range(ST):
                nc.tensor.matmul(rsT_ps[:, j:j + 1],
                                 acc[:, j * 128:(j + 1) * 128],
                                 onesb[:], start=True, stop=True)
            nc.vector.reciprocal(rcpT[:], rsT_ps[:])

            # oT psum -> SBUF bf16 (4 slices alternating scalar/DVE)
            for n in range(4):
                sl = slice(n * 512, (n + 1) * 512)
                if n % 2 == 0:
                    nc.scalar.activation(oT_sb[:, sl], oT_ps[:, sl],
                                         AF.Identity, scale=1.0)
                else:
                    nc.vector.tensor_copy(oT_sb[:, sl], oT_ps[:, sl])
            poT_cm.__exit__(None, None, None)

            # transpose blocks in groups of 4, one wide broadcast-multiply
            # per group, one batched store DMA per group
            with tc.tile_pool(name="pft", bufs=2, space="PSUM",
                              side="left") as pft:
                for g in range(4):
                    ftw = pft.tile([128, 512], bf16, tag="ftw")
                    for i in range(4):
                        st = g * 4 + i
                        nc.tensor.transpose(
                            ftw[:, i * 128:(i + 1) * 128],
                            oT_sb[:, st * 128:(st + 1) * 128], identb[:])
                    gsl = slice(g * 512, (g + 1) * 512)
                    nc.vector.tensor_mul(
                        stage[:, gsl].rearrange("p (t h) -> p t h", t=4),
                        ftw[:].rearrange("p (t h) -> p t h", t=4),
                        rcpT[:, g * 4:(g + 1) * 4, None]
                        .broadcast_to([128, 4, H]))
                    eng = nc.sync if g % 2 == 0 else nc.scalar
                    eng.dma_start(
                        out=out_d[g * 512:(g + 1) * 512, :]
                        .rearrange("(t p) d -> p t d", p=128),
                        in_=stage[:, gsl].rearrange("p (t h) -> p t h", t=4))
            pf_cm.__exit__(None, None, None)

    nc.finalize()
    return nc


def _get_nc():
    if "nc" not in _CACHE:
        _CACHE["nc"] = _build()
    return _CACHE["nc"]


def make_in_maps(x, Wq, bq, Wk, bk, Wv, bv):
    import ml_dtypes

    bf = ml_dtypes.bfloat16
    x = np.asarray(x, dtype=np.float32)
    eye = np.eye(128, dtype=np.float32)
    shared = {
        "identb": eye.astype(bf).view(np.uint16),
        "Wq": np.asarray(Wq, np.float32).astype(bf).view(np.uint16),
        "bq": np.asarray(bq, np.float32),
        "Wk": np.asarray(Wk, np.float32).astype(bf).view(np.uint16),
        "bk": np.asarray(bk, np.float32),
        "Wv": np.asarray(Wv, np.float32).astype(bf).view(np.uint16),
        "bv": np.asarray(bv, np.float32),
    }
    in_maps = []
    for b in range(NCORES):
        xTb = np.ascontiguousarray(x[b].T).astype(bf).view(np.uint16)
        in_maps.append({"xT": xTb, **shared})
    return in_maps


def kernel(x, enc_output, Wq, bq, Wk, bk, Wv, bv):
    from concourse.bass_utils import run_bass_kernel_spmd

    nc = _get_nc()
    in_maps = make_in_maps(x, Wq, bq, Wk, bk, Wv, bv)
    res = run_bass_kernel_spmd(nc, in_maps, list(range(NCORES)))
    out = np.stack([res.results[b]["out"] for b in range(NCORES)], axis=0)
    return out.astype(np.float32)
